# revision 1
# baseline (speedup 1.0000x reference)
"""Trainium2 Bass kernel for nn_AdaptiveEmbeddingI2T (8 NeuronCores).

Math (algebraically collapsed from the reference):
  img_repr r_i = mean_R img[i];  gamma/beta = MLP(r_i)
  pm_j = masked-mean_t cap[j]  (weights 1/len, BN folded out)
  BN stats: mean,var over all (B,T) per feature
  With gi = (1+gamma)*invstd, diff = beta - gi*mean:
    txt_ij = gi*pm_j + diff   (per feature)
    num       = P1.pm_j + t_i        (P1 = r o gi,   t = r.diff)
    ||txt||^2 = P2.pm2_j + P3x2.pm_j + s_i
                (P2 = gi^2, P3x2 = 2 gi o diff, s = ||diff||^2)
    sim[i,j] = invn_i * num / (sqrt(||txt||^2) + 1e-8),  invn = 1/(||r||+1e-8)
  Output is sim.T  (caption-major).

Sharding: images and captions both split 8 ways (32 each per core). One
AllGather exchanges each core's (128, 272) block = 8 d-chunks of
[32 pm columns | sum(x) | sum(x^2)] in transposed (d-on-partition) layout.
"""

import os
import sys

sys.path.insert(0, "/opt/trn_rl_repo")

import numpy as np
import ml_dtypes

BF16_NP = ml_dtypes.bfloat16

from concourse import bacc, bass, mybir, tile
from concourse.tile_rust import add_dep_helper
from concourse.alu_op_type import AluOpType
from concourse.bass_utils import run_bass_kernel_spmd

NCORES = 8
F32_INPUTS = bool(int(os.environ.get("KERNEL_F32_INPUTS", "0")))
B, T, R, D, H = 256, 72, 36, 1024, 128
BL = B // NCORES            # 32 images / captions per core
CAP_ROWS = BL * T           # 2304
IMG_ROWS = BL * R           # 1152
NT_CAP = CAP_ROWS // 128    # 18 cap row-tiles
NT_IMG = IMG_ROWS // 128    # 9 img row-tiles
CAP_SLAB = 3                # row-tiles per cap DMA slab
IMG_SLAB = 3
N_CSLAB = NT_CAP // CAP_SLAB  # 6
N_ISLAB = NT_IMG // IMG_SLAB  # 3
NCH = D // 128              # 8 feature chunks
NBT = float(B * T)          # BN sample count
EPS_BN = 1e-5
EPS_L2 = 1e-8

F32 = mybir.dt.float32
BF16 = mybir.dt.bfloat16
Act = mybir.ActivationFunctionType


def _build_kernel():
    nc = bacc.Bacc(None, num_devices=NCORES, num_swdge_queues=2)

    p = {}

    def param(name, shape, dt=F32):
        p[name] = nc.declare_dram_parameter(name, list(shape), dt, isOutput=False)
        return p[name]

    in_dt = F32 if F32_INPUTS else BF16
    param("cap", (CAP_ROWS, D), in_dt)
    param("img", (IMG_ROWS, D), in_dt)
    param("wsel", (128, NT_CAP * 17), BF16)   # pretiled: (p, t*17+c)
    param("simg", (128, NT_IMG * BL), BF16)   # pretiled: (p, t*32+c)
    param("wg1", (128, D), BF16)              # pretiled chunk-major (p, c*128+h)
    param("wb1", (128, D), BF16)
    param("wg2", (H, D), BF16)
    param("wb2", (H, D), BF16)
    param("bg1", (H, 1))
    param("bb1", (H, 1))
    param("bg2p1", (128, NCH))                # pretiled (p, c); = bg2 + 1
    param("bb2", (128, NCH))
    param("ident", (128, 128))
    out = nc.declare_dram_parameter("out", [BL, B], F32, isOutput=True)

    dbg = None
    if os.environ.get("KERNEL_DEBUG"):
        dbg = {
            "dbg_gsrc": nc.declare_dram_parameter("dbg_gsrc", [128, NCH * BL + 32], BF16, isOutput=True),
            "dbg_stat": nc.declare_dram_parameter("dbg_stat", [128, 16], F32, isOutput=True),
            "dbg_pool": nc.declare_dram_parameter("dbg_pool", [128, NCH * B], F32, isOutput=True),
            "dbg_rsb": nc.declare_dram_parameter("dbg_rsb", [BL, D], F32, isOutput=True),
        }

    with tile.TileContext(nc) as tc:
        _body(nc, tc, p, out, dbg)

    nc.compile()
    return nc


def _body(nc, tc, p, out, dbg=None):
    rg = [list(range(NCORES))]

    with (
        tc.tile_pool(name="capio", bufs=3) as capio,
        tc.tile_pool(name="persist", bufs=1) as pers,
        tc.tile_pool(name="coeff", bufs=1) as coeff,
        tc.tile_pool(name="pool_big", bufs=1) as poolbig,
        tc.tile_pool(name="dram", bufs=1, space="DRAM") as dram,
    ):
        # small persistent inputs: identity + selections (sync), weights +
        # biases (gpsimd queue so they don't contend with bulk cap/img DMAs)
        ident_sb = pers.tile([128, 128], F32)
        nc.gpsimd.dma_start(ident_sb[:, :], p["ident"][:, :])

        ws_all = pers.tile([128, NT_CAP * 17], BF16)
        nc.sync.dma_start(ws_all[:, :], p["wsel"][:, :])
        si_all = pers.tile([128, NT_IMG * BL], BF16)
        nc.gpsimd.dma_start(si_all[:, :], p["simg"][:, :])

        wg1_sb = pers.tile([128, D], BF16)  # chunk-major (p, c*128+h)
        nc.gpsimd.dma_start(wg1_sb[:, :], p["wg1"][:, :])
        wb1_sb = pers.tile([128, D], BF16)
        nc.gpsimd.dma_start(wb1_sb[:, :], p["wb1"][:, :])
        wg2_sb = pers.tile([128, D], BF16)  # natural (h, d)
        nc.gpsimd.dma_start(wg2_sb[:, :], p["wg2"][:, :])
        wb2_sb = pers.tile([128, D], BF16)
        nc.gpsimd.dma_start(wb2_sb[:, :], p["wb2"][:, :])
        bg1_sb = pers.tile([128, 1], F32)
        nc.gpsimd.dma_start(bg1_sb[:, :], p["bg1"][:, :])
        bb1_sb = pers.tile([128, 1], F32)
        nc.gpsimd.dma_start(bb1_sb[:, :], p["bb1"][:, :])
        bg2p1_sb = pers.tile([128, NCH], F32)
        nc.gpsimd.dma_start(bg2p1_sb[:, :], p["bg2p1"][:, :])
        bb2_sb = pers.tile([128, NCH], F32)
        nc.gpsimd.dma_start(bb2_sb[:, :], p["bb2"][:, :])

        pmsb = pers.tile([33, D], F32)
        s2row = pers.tile([1, D], F32)
        # gather block: cols 0:256 = pm (bf16), cols 256:288 = stats region
        # (8 sum(x) + 8 sum(x^2) stored as f32 in the bf16 tile via bitcast)
        gsrc = pers.tile([128, NCH * BL + 32], BF16)
        rsb = pers.tile([BL, D], F32)
        rT = pers.tile([128, NCH * BL], BF16)

        cap_v = p["cap"].ap().rearrange("(t p) d -> p t d", t=NT_CAP)
        img_v = p["img"].ap().rearrange("(t p) d -> p t d", t=NT_IMG)

        with (
            tc.tile_pool(name="ps_cap", bufs=1, space="PSUM") as ps_cap,
            tc.tile_pool(name="ps_img", bufs=1, space="PSUM") as ps_img,
            tc.tile_pool(name="ps_tr", bufs=2, space="PSUM") as ps_tr,
        ):
            # HAM warm-up: keep the PE busy while the first cap slab is in
            # flight so real matmuls run at 2.4 GHz instead of 1.2.
            for w in range(56):
                wp = ps_tr.tile([128, 16], F32, tag="tp")
                nc.tensor.matmul(wp[0:17, 0:16], ws_all[:, 0:17],
                                 ws_all[:, 0:16], start=True, stop=True)

            # ===== Phase A: captions -> pm (masked mean) + BN partials =====
            # T-form: stationary = capb d-chunk (128 rows, 128 d), moving =
            # wsel (16 caption cols + ones col) -> psum (128 d, 17) per chunk,
            # pm lands already d-on-partition; col 16 = sum(x).
            # Caption rows are block-diagonal (16*72 = 9*128): halves use
            # psum rows 0..15 via the b%16 column mapping.
            # sum(x^2): b-major via ws2 = [zeros|ones] -> psum_S rows [0|1].
            psum_T0 = ps_cap.tile([128, 4 * 17], F32, tag="t0")  # chunks 0-3
            psum_T1 = ps_cap.tile([128, 4 * 17], F32, tag="t1")  # chunks 4-7
            psum_S0 = ps_cap.tile([2, 512], F32, tag="s0")
            psum_S1 = ps_cap.tile([2, 512], F32, tag="s1")
            ws2 = pers.tile([128, 2], BF16)   # [ones | zeros]: sumsq in row 0
            nc.vector.memset(ws2[:, 0:1], 1.0)
            nc.vector.memset(ws2[:, 1:2], 0.0)
            gstat = gsrc[:, NCH * BL:NCH * BL + 32].bitcast(F32)   # (128, 16)
            gstatA = pers.tile([128, 16], F32)
            s2row = pers.tile([1, D], F32)

            HT = NT_CAP // 2  # 9 tiles per half
            bank_start = [None, None]
            for half in range(2):
                for s in range(N_CSLAB // 2 * half, N_CSLAB // 2 * (half + 1)):
                    t0 = s * CAP_SLAB
                    if F32_INPUTS:
                        cap_s = capio.tile([128, CAP_SLAB * D], F32, tag="cap")
                        dma_eng = nc.sync if s % 2 == 0 else nc.scalar
                        dma_eng.dma_start(
                            cap_s[:, :].rearrange("p (t d) -> p t d", t=CAP_SLAB),
                            cap_v[:, t0:t0 + CAP_SLAB, :])
                        capb = capio.tile([128, CAP_SLAB * D], BF16, tag="capb")
                        nc.vector.tensor_copy(capb[:, :], cap_s[:, :])
                    else:
                        capb = capio.tile([128, CAP_SLAB * D], BF16, tag="capb")
                        dma_eng = nc.sync if s % 2 == 0 else nc.scalar
                        dma_eng.dma_start(
                            capb[:, :].rearrange("p (t d) -> p t d", t=CAP_SLAB),
                            cap_v[:, t0:t0 + CAP_SLAB, :])
                        cap_s = capb
                    sq_s = capio.tile([128, CAP_SLAB * D], BF16, tag="sq")
                    nc.scalar.activation(sq_s[:, :], cap_s[:, :], Act.Square)
                    for i in range(CAP_SLAB):
                        t = t0 + i
                        st = (t % HT == 0)
                        sp = (t % HT == HT - 1)
                        stS = (t == 0)
                        spS = (t == NT_CAP - 1)
                        ws_t = ws_all[:, t * 17:(t + 1) * 17]
                        for c in range(NCH):
                            pt = psum_T0 if c < 4 else psum_T1
                            col = (c % 4) * 17
                            # one accumulation epoch per BANK per half:
                            # start only on the chronologically-first matmul
                            # (start clears has_written for the whole bank),
                            # stop only on the last; explicit edges pin the
                            # start matmul first within its bank epoch.
                            mm = nc.tensor.matmul(
                                pt[:, col:col + 17],
                                capb[:, i * D + c * 128:i * D + (c + 1) * 128],
                                ws_t, start=(st and c % 4 == 0),
                                stop=(sp and c % 4 == 3),
                                skip_group_check=True)
                            if st and c % 4 == 0:
                                bank_start[c // 4] = mm
                            elif st:
                                add_dep_helper(mm.ins, bank_start[c // 4].ins,
                                               reason="bank epoch start first")
                        nc.tensor.matmul(psum_S0[:, :], ws2[:, :],
                                         sq_s[:, i * D:i * D + 512],
                                         start=stS, stop=spS)
                        nc.tensor.matmul(psum_S1[:, :], ws2[:, :],
                                         sq_s[:, i * D + 512:(i + 1) * D],
                                         start=stS, stop=spS)

                # pack this half: strided copies, no transposes
                r0 = 16 * half
                for q, pt in enumerate((psum_T0, psum_T1)):
                    src_pm = pt[:, :].rearrange("p (c j) -> p c j", c=4)[:, :, 0:16]
                    dst_pm = gsrc[:, 0:NCH * BL] \
                        .rearrange("p (c j) -> p c j", c=NCH) \
                        [:, 4 * q:4 * q + 4, r0:r0 + 16]
                    src_sx = pt[:, :].rearrange("p (c j) -> p c j", c=4)[:, :, 16:17]
                    dststat = (gstatA if half == 0 else gstat)
                    dst_sx = dststat[:, 4 * q:4 * q + 4].unsqueeze(2)
                    if q == 0:
                        nc.scalar.copy(dst_pm, src_pm)
                        nc.scalar.copy(dst_sx, src_sx)
                    else:
                        nc.vector.tensor_copy(dst_pm, src_pm)
                        nc.vector.tensor_copy(dst_sx, src_sx)
            # total sum(x) = half0 + half1 (cols 0:8); then sum(x^2) pack
            nc.vector.tensor_tensor(gstat[:, 0:8], gstat[:, 0:8],
                                    gstatA[:, 0:8], AluOpType.add)
            nc.scalar.copy(s2row[0:1, 0:512], psum_S0[0:1, :])
            nc.vector.tensor_copy(s2row[0:1, 512:1024], psum_S1[0:1, :])
            for c in range(NCH):
                tp2 = ps_tr.tile([128, 16], F32, tag="tp")
                nc.tensor.transpose(tp2[:, 0:1], s2row[:, c * 128:(c + 1) * 128],
                                    ident_sb[0:1, 0:1])
                if c % 2 == 0:
                    nc.scalar.copy(gstat[:, 8 + c:9 + c], tp2[:, 0:1])
                else:
                    nc.vector.tensor_copy(gstat[:, 8 + c:9 + c], tp2[:, 0:1])

            # ===== Phase B: AllGather (pm blocks + BN partials) =====
            cc_in = dram.tile([128, NCH * BL + 32], BF16)
            cc_out = dram.tile([128 * NCORES, NCH * BL + 32], BF16,
                               addr_space="Shared")
            nc.gpsimd.dma_start(cc_in[:, :], gsrc[:, :])
            nc.gpsimd.collective_compute(
                "AllGather", AluOpType.bypass, replica_groups=rg,
                ins=[cc_in.opt()], outs=[cc_out.opt()],
            )

            # ===== Phase C: images (independent of collective) =====
            psum_I0 = ps_img.tile([BL, 512], F32, tag="i0")
            psum_I1 = ps_img.tile([BL, 512], F32, tag="i1")
            for s in range(N_ISLAB):
                t0 = s * IMG_SLAB
                if F32_INPUTS:
                    img_s = capio.tile([128, IMG_SLAB * D], F32, tag="cap")
                    dma_eng = nc.sync if s % 2 == 0 else nc.scalar
                    dma_eng.dma_start(
                        img_s[:, :].rearrange("p (t d) -> p t d", t=IMG_SLAB),
                        img_v[:, t0:t0 + IMG_SLAB, :])
                    imgb = capio.tile([128, IMG_SLAB * D], BF16, tag="capb")
                    nc.vector.tensor_copy(imgb[:, :], img_s[:, :])
                else:
                    imgb = capio.tile([128, IMG_SLAB * D], BF16, tag="capb")
                    dma_eng = nc.sync if s % 2 == 0 else nc.scalar
                    dma_eng.dma_start(
                        imgb[:, :].rearrange("p (t d) -> p t d", t=IMG_SLAB),
                        img_v[:, t0:t0 + IMG_SLAB, :])
                for i in range(IMG_SLAB):
                    t = t0 + i
                    st, sp = (t == 0), (t == NT_IMG - 1)
                    si_t = si_all[:, t * BL:(t + 1) * BL]
                    nc.tensor.matmul(psum_I0[:, :], si_t,
                                     imgb[:, i * D:i * D + 512],
                                     start=st, stop=sp)
                    nc.tensor.matmul(psum_I1[:, :], si_t,
                                     imgb[:, i * D + 512:(i + 1) * D],
                                     start=st, stop=sp)
            nc.scalar.copy(rsb[:, 0:512], psum_I0[:, :])
            nc.scalar.copy(rsb[:, 512:1024], psum_I1[:, :])

            # transpose img_repr to chunk-major (128, 8*32) bf16
            for c in range(NCH):
                tp = ps_tr.tile([128, 34], F32, tag="tp")
                nc.tensor.transpose(tp[:, 0:BL], rsb[:, c * 128:(c + 1) * 128],
                                    ident_sb[0:BL, 0:BL])
                nc.scalar.copy(rT[:, c * BL:(c + 1) * BL], tp[:, 0:BL])

        if dbg:
            nc.gpsimd.dma_start(dbg["dbg_rsb"][:, :], rsb[:, :])

        gT = coeff.tile([128, NCH * BL], BF16)
        bT = coeff.tile([128, NCH * BL], BF16)
        with tc.tile_pool(name="ps_mlp", bufs=2, space="PSUM") as ps_mlp:
            # MLP hidden: (128h, 32)
            psum_hg = ps_mlp.tile([128, BL], F32, tag="h")
            psum_hb = ps_mlp.tile([128, BL], F32, tag="h")
            for c in range(NCH):
                st, sp = (c == 0), (c == NCH - 1)
                nc.tensor.matmul(psum_hg[:, :], wg1_sb[:, c * 128:(c + 1) * 128],
                                 rT[:, c * BL:(c + 1) * BL], start=st, stop=sp)
                nc.tensor.matmul(psum_hb[:, :], wb1_sb[:, c * 128:(c + 1) * 128],
                                 rT[:, c * BL:(c + 1) * BL], start=st, stop=sp)
            hg = pers.tile([128, BL], BF16)
            nc.scalar.activation(hg[:, :], psum_hg[:, :], Act.Relu, bias=bg1_sb[:, 0:1])
            hb = pers.tile([128, BL], BF16)
            nc.scalar.activation(hb[:, :], psum_hb[:, :], Act.Relu, bias=bb1_sb[:, 0:1])

            # gamma+1 / beta, chunk-major T layout (128, 8*32)
            for c in range(NCH):
                pg = ps_mlp.tile([128, BL], F32, tag="gb")
                nc.tensor.matmul(pg[:, :], wg2_sb[:, c * 128:(c + 1) * 128],
                                 hg[:, :], start=True, stop=True)
                nc.scalar.activation(gT[:, c * BL:(c + 1) * BL], pg[:, :], Act.Identity,
                                     bias=bg2p1_sb[:, c:c + 1])
                pb = ps_mlp.tile([128, BL], F32, tag="gb")
                nc.tensor.matmul(pb[:, :], wb2_sb[:, c * 128:(c + 1) * 128],
                                 hb[:, :], start=True, stop=True)
                nc.scalar.activation(bT[:, c * BL:(c + 1) * BL], pb[:, :], Act.Identity,
                                     bias=bb2_sb[:, c:c + 1])

        # ===== image-side coefficients (no BN dependence; runs during
        # the collective window) =====
        P1 = coeff.tile([128, NCH * BL], BF16)
        nc.vector.tensor_tensor(P1[:, :], rT[:, :], gT[:, :], AluOpType.mult)
        P2 = coeff.tile([128, NCH * BL], BF16)
        nc.vector.tensor_tensor(P2[:, :], gT[:, :], gT[:, :], AluOpType.mult)
        P3x2 = coeff.tile([128, NCH * BL], BF16)
        nc.vector.tensor_tensor(P3x2[:, :], gT[:, :], bT[:, :], AluOpType.mult)
        nc.vector.tensor_scalar(P3x2[:, :], P3x2[:, :], 2.0, None, AluOpType.mult)
        rb = coeff.tile([128, NCH * BL], BF16)
        nc.vector.tensor_tensor(rb[:, :], rT[:, :], bT[:, :], AluOpType.mult)
        b2 = coeff.tile([128, NCH * BL], BF16)
        nc.vector.tensor_tensor(b2[:, :], bT[:, :], bT[:, :], AluOpType.mult)
        r2 = coeff.tile([128, NCH * BL], BF16)
        nc.vector.tensor_tensor(r2[:, :], rT[:, :], rT[:, :], AluOpType.mult)

        ones_sb = pers.tile([128, 1], BF16)
        nc.vector.memset(ones_sb[:, :], 1.0)
        epsbn = pers.tile([128, 1], F32)
        nc.vector.memset(epsbn[:, :], EPS_BN)

        with tc.tile_pool(name="ps_fin", bufs=1, space="PSUM") as ps_fin:
            # per-image scalars via ones-matmuls -> (32, 1) psums
            # t = sum r*b, s = ||b||^2, r2 = ||r||^2
            psum_t = ps_fin.tile([BL, 1], F32, tag="sct")
            psum_s = ps_fin.tile([BL, 1], F32, tag="scs")
            psum_r2 = ps_fin.tile([BL, 1], F32, tag="scr")
            for c in range(NCH):
                st, sp = (c == 0), (c == NCH - 1)
                sl = slice(c * BL, (c + 1) * BL)
                nc.tensor.matmul(psum_t[:, :], rb[:, sl], ones_sb[:, :],
                                 start=st, stop=sp)
                nc.tensor.matmul(psum_s[:, :], b2[:, sl], ones_sb[:, :],
                                 start=st, stop=sp)
                nc.tensor.matmul(psum_r2[:, :], r2[:, sl], ones_sb[:, :],
                                 start=st, stop=sp)
            t_col = pers.tile([BL, 1], F32)
            nc.scalar.copy(t_col[:, :], psum_t[:, :])
            s_col = pers.tile([BL, 1], F32)
            nc.scalar.copy(s_col[:, :], psum_s[:, :])
            nrm = pers.tile([BL, 1], F32)
            nc.scalar.activation(nrm[:, :], psum_r2[:, :], Act.Sqrt)
            nrme = pers.tile([BL, 1], F32)
            nc.vector.tensor_scalar(nrme[:, :], nrm[:, :], EPS_L2, None, AluOpType.add)
            invn = pers.tile([BL, 1], F32)
            nc.vector.reciprocal(invn[:, :], nrme[:, :])

            # ===== Phase D: post-collective =====
            # stats from all ranks in one DMA, then DVE-reduce
            statall = pers.tile([128, NCORES * 16], F32)
            nc.scalar.dma_start(
                statall[:, :].rearrange("p (k f) -> p k f", k=NCORES),
                cc_out[:, NCH * BL:NCH * BL + 32].bitcast(F32)
                .rearrange("(k p) f -> p k f", k=NCORES))
            statacc = pers.tile([128, 16], F32)
            nc.vector.tensor_tensor(statacc[:, :], statall[:, 0:16],
                                    statall[:, 16:32], AluOpType.add)
            for k in range(2, NCORES):
                nc.vector.tensor_tensor(statacc[:, :], statacc[:, :],
                                        statall[:, k * 16:(k + 1) * 16],
                                        AluOpType.add)

            # BN stats: invT = 1/sqrt((S2 - S1^2/N)/N + eps) via fused
            # ACT Sqrt(scale=1/N); meanT = S1/N computed off-path.
            msqn = pers.tile([128, NCH], F32)
            nc.vector.tensor_tensor(msqn[:, :], statacc[:, 0:8], statacc[:, 0:8],
                                    AluOpType.mult)
            nc.vector.tensor_scalar(msqn[:, :], msqn[:, :], 1.0 / NBT, None,
                                    AluOpType.mult)
            varn = pers.tile([128, NCH], F32)
            nc.vector.tensor_tensor(varn[:, :], statacc[:, 8:16], msqn[:, :],
                                    AluOpType.subtract)
            sd = pers.tile([128, NCH], F32)
            nc.scalar.activation(sd[:, :], varn[:, :], Act.Sqrt,
                                 bias=epsbn[:, 0:1], scale=1.0 / NBT)
            invT = pers.tile([128, NCH], F32)
            nc.vector.reciprocal(invT[:, :], sd[:, :])
            meanT = pers.tile([128, NCH], F32)
            nc.vector.tensor_scalar(meanT[:, :], statacc[:, 0:8], 1.0 / NBT, None,
                                    AluOpType.mult)

            # pooled raw (128, 8ranks*256): natural k-major layout, one DMA
            poolraw = poolbig.tile([128, NCH * B], BF16)
            nc.gpsimd.dma_start(
                poolraw[:, :].rearrange("p (k cj) -> p k cj", k=NCORES),
                cc_out[:, 0:NCH * BL]
                .rearrange("(k p) cj -> p k cj", k=NCORES))

            if dbg:
                nc.gpsimd.dma_start(dbg["dbg_stat"][:, :], statacc[:, :])

            # BN-normalize + bf16-cast + k-major -> chunk-major permute,
            # one tensor_scalar per chunk: (x - mean_c) * inv_c
            praw_v = poolraw[:, :].rearrange("p (k c j) -> p k c j",
                                             k=NCORES, c=NCH)
            pooledTb = poolbig.tile([128, NCH * B], BF16)
            pTb_v = pooledTb[:, :].rearrange("p (c k j) -> p c k j",
                                             c=NCH, k=NCORES)
            pooled2Tb = poolbig.tile([128, NCH * B], BF16)
            for c in range(NCH):
                jsl = slice(c * B, (c + 1) * B)
                nc.vector.tensor_scalar(pTb_v[:, c, :, :], praw_v[:, :, c, :],
                                        meanT[:, c:c + 1], invT[:, c:c + 1],
                                        AluOpType.subtract, AluOpType.mult)
                nc.vector.tensor_tensor(pooled2Tb[:, jsl], pooledTb[:, jsl],
                                        pooledTb[:, jsl], AluOpType.mult)

            # HAM warm-up during the collective wait (PE idle ~40us there,
            # otherwise the tail matmuls run at half clock)
            for w in range(90):
                wp = ps_fin.tile([BL, 16], F32, tag="warm")
                nc.tensor.matmul(wp[0:17, 0:16], ws_all[:, 0:17],
                                 ws_all[:, 0:16], start=True, stop=True)

            # ===== Phase E: final matmuls + epilogue =====
            psum_A = ps_fin.tile([BL, B], F32, tag="A")
            psum_D = ps_fin.tile([BL, B], F32, tag="Dd")
            for c in range(NCH):
                st, sp = (c == 0), (c == NCH - 1)
                isl = slice(c * BL, (c + 1) * BL)
                jsl = slice(c * B, (c + 1) * B)
                nc.tensor.matmul(psum_A[:, :], P1[:, isl], pooledTb[:, jsl],
                                 start=st, stop=sp)
                nc.tensor.matmul(psum_D[:, :], P2[:, isl], pooled2Tb[:, jsl],
                                 start=st, stop=False)
                nc.tensor.matmul(psum_D[:, :], P3x2[:, isl], pooledTb[:, jsl],
                                 start=False, stop=sp)

            den = pers.tile([BL, B], F32)
            nc.scalar.activation(den[:, :], psum_D[:, :], Act.Sqrt,
                                 bias=s_col[:, 0:1])
            rec = pers.tile([BL, B], F32)
            nc.vector.reciprocal(rec[:, :], den[:, :])
            num = pers.tile([BL, B], F32)
            nc.vector.tensor_scalar(num[:, :], psum_A[:, :], t_col[:, 0:1],
                                    invn[:, 0:1], AluOpType.add, AluOpType.mult)
            sim_sb = pers.tile([BL, B], F32)
            nc.vector.tensor_tensor(sim_sb[:, :], num[:, :], rec[:, :],
                                    AluOpType.mult)
            nc.gpsimd.dma_start(out[:, :], sim_sb[:, :])


_NC_CACHE = None


def _get_nc():
    global _NC_CACHE
    if _NC_CACHE is None:
        _NC_CACHE = _build_kernel()
    return _NC_CACHE


def _install_ntff_shim():
    """Expose the axon NTFF profile hook so trace=True works (best effort)."""
    import types
    if "antenv.axon_hooks" in sys.modules:
        return
    try:
        mod = types.ModuleType("antenv.axon_hooks")
        state = {"hook": None}
        mod.set_axon_ntff_profile_hook = lambda h: state.__setitem__("hook", h)
        mod.get_axon_ntff_profile_hook = lambda: state["hook"]
        sys.modules["antenv.axon_hooks"] = mod
        import antenv
        antenv.axon_hooks = mod
        from trn_agent_boot.trn_boot import _ntff_profile_via_ctypes
        hook = _ntff_profile_via_ctypes("/opt/axon/libaxon_pjrt.so")
        mod.set_axon_ntff_profile_hook(hook)
    except Exception as e:  # profiling is optional; never break the run
        print(f"ntff shim unavailable: {e}", file=sys.stderr)


last_exec_time_ns = None
last_results = None


def kernel(img_embed, cap_embed, lens, Wg1, bg1, Wg2, bg2, Wb1, bb1, Wb2, bb2):
    global last_exec_time_ns, last_results
    img_embed = np.ascontiguousarray(np.asarray(img_embed, dtype=np.float32))
    cap_embed = np.ascontiguousarray(np.asarray(cap_embed, dtype=np.float32))
    lens = np.asarray(lens).astype(np.int64)

    # host-side prep: per-core shards + selection/mask weight matrices
    ident = np.eye(128, dtype=np.float32)
    in_maps = []
    for k in range(NCORES):
        jsl = slice(k * BL, (k + 1) * BL)
        cap_k = cap_embed[jsl].reshape(CAP_ROWS, D)
        img_k = img_embed[jsl].reshape(IMG_ROWS, D)
        lens_k = lens[jsl]
        # wsel[(b,t), c] = (t < len_b)/len_b at col b%16; col 16 = ones
        wsel = np.zeros((BL, T, 17), dtype=np.float32)
        for b in range(BL):
            wsel[b, : lens_k[b], b % 16] = 1.0 / float(lens_k[b])
        wsel[:, :, 16] = 1.0
        simg = np.zeros((BL, R, BL), dtype=np.float32)
        for b in range(BL):
            simg[b, :, b] = 1.0 / R
        wsel_t = np.ascontiguousarray(
            wsel.reshape(NT_CAP, 128, 17).transpose(1, 0, 2)
            .reshape(128, NT_CAP * 17)).astype(BF16_NP)
        simg_t = np.ascontiguousarray(
            simg.reshape(NT_IMG, 128, BL).transpose(1, 0, 2)
            .reshape(128, NT_IMG * BL)).astype(BF16_NP)
        wg1_t = np.ascontiguousarray(
            np.asarray(Wg1, np.float32).reshape(NCH, 128, H).transpose(1, 0, 2)
            .reshape(128, D)).astype(BF16_NP)
        wb1_t = np.ascontiguousarray(
            np.asarray(Wb1, np.float32).reshape(NCH, 128, H).transpose(1, 0, 2)
            .reshape(128, D)).astype(BF16_NP)
        if not F32_INPUTS:
            cap_k = cap_k.astype(BF16_NP)
            img_k = img_k.astype(BF16_NP)
        in_maps.append({
            "cap": cap_k,
            "img": img_k,
            "wsel": wsel_t,
            "simg": simg_t,
            "wg1": wg1_t,
            "wb1": wb1_t,
            "wg2": np.ascontiguousarray(Wg2).astype(BF16_NP),
            "wb2": np.ascontiguousarray(Wb2).astype(BF16_NP),
            "bg1": np.asarray(bg1, dtype=np.float32).reshape(H, 1),
            "bb1": np.asarray(bb1, dtype=np.float32).reshape(H, 1),
            "bg2p1": np.ascontiguousarray(
                (np.asarray(bg2, np.float32) + 1.0).reshape(NCH, 128).T),
            "bb2": np.ascontiguousarray(
                np.asarray(bb2, np.float32).reshape(NCH, 128).T),
            "ident": ident,
        })

    nc = _get_nc()
    trace = bool(int(os.environ.get("BASS_KERNEL_TRACE", "0")))
    if trace:
        _install_ntff_shim()
    res = run_bass_kernel_spmd(nc, in_maps, list(range(NCORES)), trace=trace)
    last_exec_time_ns = res.exec_time_ns
    last_results = res

    sim_ij = np.concatenate([res.results[k]["out"] for k in range(NCORES)], axis=0)
    return np.ascontiguousarray(sim_ij.T)

